# revision 7
# baseline (speedup 1.0000x reference)
"""GraphQLayer kernel v5: symmetric half-Gram (triangle) scheme.

Ownership: core c owns global 128-row blocks {8t+c, t=0..15}. Each core
computes only tiles (t, jt) with jt >= 2t (conservative SPMD-uniform bound
for floor((8t+c)/4)); sub/straddle-diagonal parts are zeroed by per-core
tmask input data, keeping the program identical across cores.

Per tile [128 rows x 512 cols]:
  PE    Gram fp32 (exact, bitwise-reference-faithful)
  ACT   Square -> sq (exact fl32(G^2)) in SBUF
  DVE   sqm = sq * tmask (straddle slots only)
        row-stt: (sq >= th) * s_j  accum -> row-partials (j >= i half)
  GPS   mask16 = (sq >= th); srt2 = mask16 * s_m (per-block broadcast)
  PE    colp[1,512] += ones.T @ srt2  (col-partials = the j < i half,
        accumulated over t per jt in PSUM)

Outputs: row-partials [2048] (owned rows) + col-partials [16384].
Host combines during unshard: agg[i] = rowpart[i] + sum_c colpart_c[i]
- 2*s16[i] (diagonal counted once in each half), then the rank-1
expansion out = agg*wsum + b.
"""

import sys
from contextlib import ExitStack

import numpy as np

sys.path.insert(0, "/opt/trn_rl_repo")

import concourse.bass as bass  # noqa: E402
import concourse.tile as tile  # noqa: E402
from concourse import bacc, mybir  # noqa: E402
from concourse.bass_utils import run_bass_kernel_spmd  # noqa: E402

N, D, H = 16384, 128, 64
NCORES = 8
NBLK = 128                    # global 128-row blocks
TB = 16                       # owned blocks per core
JW = 512
JT = N // JW                  # 32
THRESH = 0.85

f32 = mybir.dt.float32
f16 = mybir.dt.float16
AOP = mybir.AluOpType
AFT = mybir.ActivationFunctionType
AXL = mybir.AxisListType


def _build_kernel(_unused=None):
    import os
    repeat = int(os.environ.get("BASS_REPEAT", "1"))
    hwloop = int(os.environ.get("BASS_HWLOOP", "0"))
    nc = bacc.Bacc("TRN2", target_bir_lowering=False, debug=False,
                   num_devices=NCORES)
    xt_d = nc.dram_tensor("xt", [128, N], f32, kind="ExternalInput").ap()
    xts_d = nc.dram_tensor("xts", [128, TB * 128], f32,
                           kind="ExternalInput").ap()
    sb_d = nc.dram_tensor("sb", [128, N], f16, kind="ExternalInput").ap()
    scols_d = nc.dram_tensor("scols", [128, TB], f16,
                             kind="ExternalInput").ap()
    tmask_d = nc.dram_tensor("tmask", [128, 2 * JW], f32,
                             kind="ExternalInput").ap()
    rowp_d = nc.dram_tensor("rowp", [TB * 128, 1], f32,
                            kind="ExternalOutput").ap()
    colp_d = nc.dram_tensor("colp", [1, N], f32, kind="ExternalOutput").ap()

    with tile.TileContext(nc) as tc:
        with ExitStack() as ctx:
            nc2 = tc.nc
            cst = ctx.enter_context(tc.tile_pool(name="cst", bufs=1))
            gp = ctx.enter_context(tc.tile_pool(name="gp", bufs=5,
                                                space="PSUM"))
            cpp = ctx.enter_context(tc.tile_pool(name="cpp", bufs=2,
                                                 space="PSUM"))
            sqp = ctx.enter_context(tc.tile_pool(name="sqp", bufs=4))
            mkp = ctx.enter_context(tc.tile_pool(name="mkp", bufs=5))
            s2p = ctx.enter_context(tc.tile_pool(name="s2p", bufs=3))
            fin = ctx.enter_context(tc.tile_pool(name="finp", bufs=2))

            xts_t = cst.tile([128, TB * 128], f32, tag="xts")
            nc2.sync.dma_start(out=xts_t[:], in_=xts_d[:])
            scols_t = cst.tile([128, TB], f16, tag="scols")
            nc2.sync.dma_start(out=scols_t[:], in_=scols_d[:])
            tmask_t = cst.tile([128, 2 * JW], f32, tag="tmask")
            nc2.sync.dma_start(out=tmask_t[:], in_=tmask_d[:])
            xt_tiles, sb_tiles = [], []
            for j in range(JT):
                t_ = cst.tile([128, JW], f32, tag=f"xt{j}")
                nc2.sync.dma_start(out=t_[:], in_=xt_d[:, j * JW:(j + 1) * JW])
                xt_tiles.append(t_)
                u = cst.tile([128, JW], f16, tag=f"sb{j}")
                nc2.sync.dma_start(out=u[:], in_=sb_d[:, j * JW:(j + 1) * JW])
                sb_tiles.append(u)

            # row-partial accumulators per owned block (slot per jt)
            aggRT = [cst.tile([128, JT], f32, tag=f"agr{t}",
                               name=f"agr{t}")
                     for t in range(TB)]

            th = float(np.float32(THRESH))

            def body(tag=""):
                for jt in range(JT):
                    t_end = jt // 2 + 1
                    ts = list(range(min(t_end, TB)))
                    colps = cpp.tile([1, JW], f32, tag="colps",
                                     name=f"colps_{tag}_{jt}")
                    pend = []

                    def emit_colp(item):
                        t_, mk_ = item
                        nc2.tensor.matmul(out=colps[:],
                                          lhsT=scols_t[:, t_:t_ + 1],
                                          rhs=mk_[:],
                                          start=(t_ == ts[0]),
                                          stop=(t_ == ts[-1]))

                    for t in ts:
                        lhsT = xts_t[:, t * 128:(t + 1) * 128]
                        g = gp.tile([128, JW], f32, tag="g")
                        nc2.tensor.matmul(out=g[:], lhsT=lhsT,
                                          rhs=xt_tiles[jt][:],
                                          start=True, stop=True)
                        sq = sqp.tile([128, JW], f32, tag="sq")
                        nc2.scalar.activation(sq[:], g[:], AFT.Square)
                        if jt == 2 * t or jt == 2 * t + 1:
                            slot = jt - 2 * t
                            sqm = sqp.tile([128, JW], f32, tag="sqm")
                            nc2.vector.tensor_tensor(
                                sqm[:], sq[:],
                                tmask_t[:, slot * JW:(slot + 1) * JW],
                                AOP.mult)
                            sq = sqm
                        mk = mkp.tile([128, JW], f16, tag="mk")
                        nc2.vector.tensor_scalar(mk[:], sq[:], th, None,
                                                 AOP.is_ge)
                        srt = s2p.tile([128, JW], f16, tag="srt")
                        nc2.vector.scalar_tensor_tensor(
                            out=srt[:], in0=mk[:], scalar=1.0,
                            in1=sb_tiles[jt][:],
                            op0=AOP.mult, op1=AOP.mult,
                            accum_out=aggRT[t][:, jt:jt + 1])
                        pend.append((t, mk))
                        if len(pend) > 3:
                            emit_colp(pend.pop(0))
                    for item in pend:
                        emit_colp(item)
                    cc = fin.tile([1, JW], f32, tag="cc",
                                  name=f"cc_{tag}_{jt}")
                    nc2.scalar.copy(cc[:], colps[:])
                    nc2.sync.dma_start(
                        out=colp_d[0:1, jt * JW:(jt + 1) * JW], in_=cc[:])

                for t in range(TB):
                    lo = 2 * t
                    agg = fin.tile([128, 1], f32, tag="agg",
                                   name=f"agg_{tag}_{t}")
                    nc2.vector.tensor_reduce(agg[:], aggRT[t][:, lo:JT],
                                             axis=AXL.X, op=AOP.add)
                    nc2.sync.dma_start(out=rowp_d[t * 128:(t + 1) * 128, :],
                                       in_=agg[:])

            if hwloop > 0:
                with tc.For_i(0, hwloop):
                    body("L")
            else:
                for rep in range(repeat):
                    body(str(rep))
    nc.compile()
    return nc


_CACHE = {}


def prep_in_maps(x, W, b):
    x = np.asarray(x, dtype=np.float32)
    xt = np.ascontiguousarray(x.T)                       # [128, N]
    s16 = (x.astype(np.float64).sum(axis=1) / 128.0).astype(np.float16)
    sb = np.ascontiguousarray(np.broadcast_to(s16[None, :], (128, N)))

    in_maps = []
    for c in range(NCORES):
        blocks = [8 * t + c for t in range(TB)]
        cols = np.concatenate([np.arange(k * 128, (k + 1) * 128)
                               for k in blocks])
        xts = np.ascontiguousarray(xt[:, cols])          # [128, 2048]
        scols = np.ascontiguousarray(
            s16[cols].reshape(TB, 128).T)                # [128, TB] f16
        # tmask slots: tile (t, jt=2t) keeps q >= 128c + p;
        #              tile (t, jt=2t+1) keeps q >= 128c - 512 + p
        p = np.arange(128)[:, None]
        q = np.arange(JW)[None, :]
        tm0 = (q >= 128 * c + p).astype(np.float32)
        tm1 = (q >= 128 * c - 512 + p).astype(np.float32)
        tmask = np.ascontiguousarray(
            np.concatenate([tm0, tm1], axis=1))          # [128, 1024]
        in_maps.append({
            "xt": xt, "xts": xts, "sb": sb,
            "scols": scols, "tmask": tmask,
        })
    return in_maps


def kernel(x: np.ndarray, W: np.ndarray, b: np.ndarray,
           trace: bool = False, tmpdir: str | None = None):
    x = np.asarray(x, dtype=np.float32)
    W = np.asarray(W, dtype=np.float32)
    b = np.asarray(b, dtype=np.float32)
    in_maps = prep_in_maps(x, W, b)

    if "nc" not in _CACHE:
        _CACHE["nc"] = _build_kernel()
    nc = _CACHE["nc"]
    res = run_bass_kernel_spmd(nc, in_maps, list(range(NCORES)),
                               trace=trace, tmpdir=tmpdir)

    s16 = (x.astype(np.float64).sum(axis=1) / 128.0).astype(np.float16)
    agg = np.zeros(N, dtype=np.float64)
    for c in range(NCORES):
        r = res.results[c]
        blocks = [8 * t + c for t in range(TB)]
        rows = np.concatenate([np.arange(k * 128, (k + 1) * 128)
                               for k in blocks])
        agg[rows] += r["rowp"][:, 0].astype(np.float64)
        agg += r["colp"][0].astype(np.float64)
    # diagonal was counted once in the row half and once in the col half
    agg -= 2.0 * s16.astype(np.float64)
    wsum1 = W.astype(np.float64).sum(axis=1)
    out = (agg[:, None] * wsum1[None, :] + b.astype(np.float64)[None, :])
    if trace:
        kernel.last_results = res
    return out.astype(np.float32)


# revision 9
# speedup vs baseline: 1.1163x; 1.1163x over previous
"""GraphQLayer kernel v5: symmetric half-Gram (triangle) scheme.

Ownership: core c owns global 128-row blocks {8t+c, t=0..15}. Each core
computes only tiles (t, jt) with jt >= 2t (conservative SPMD-uniform bound
for floor((8t+c)/4)); sub/straddle-diagonal parts are zeroed by per-core
tmask input data, keeping the program identical across cores.

Per tile [128 rows x 512 cols]:
  PE    Gram fp32 (exact, bitwise-reference-faithful)
  ACT   Square -> sq (exact fl32(G^2)) in SBUF
  DVE   sqm = sq * tmask (straddle slots only)
        row-stt: (sq >= th) * s_j  accum -> row-partials (j >= i half)
  GPS   mask16 = (sq >= th); srt2 = mask16 * s_m (per-block broadcast)
  PE    colp[1,512] += ones.T @ srt2  (col-partials = the j < i half,
        accumulated over t per jt in PSUM)

Outputs: row-partials [2048] (owned rows) + col-partials [16384].
Host combines during unshard: agg[i] = rowpart[i] + sum_c colpart_c[i]
- 2*s16[i] (diagonal counted once in each half), then the rank-1
expansion out = agg*wsum + b.
"""

import sys
from contextlib import ExitStack

import numpy as np

sys.path.insert(0, "/opt/trn_rl_repo")

import concourse.bass as bass  # noqa: E402
import concourse.tile as tile  # noqa: E402
from concourse import bacc, mybir  # noqa: E402
from concourse.bass_utils import run_bass_kernel_spmd  # noqa: E402

N, D, H = 16384, 128, 64
NCORES = 8
NBLK = 128                    # global 128-row blocks
TB = 16                       # owned blocks per core
JW = 512
JT = N // JW                  # 32
THRESH = 0.85

f32 = mybir.dt.float32
f16 = mybir.dt.float16
AOP = mybir.AluOpType
AFT = mybir.ActivationFunctionType
AXL = mybir.AxisListType


def _build_kernel(_unused=None):
    import os
    repeat = int(os.environ.get("BASS_REPEAT", "1"))
    hwloop = int(os.environ.get("BASS_HWLOOP", "0"))
    nc = bacc.Bacc("TRN2", target_bir_lowering=False, debug=False,
                   num_devices=NCORES)
    xt_d = nc.dram_tensor("xt", [128, N], f32, kind="ExternalInput").ap()
    xts_d = nc.dram_tensor("xts", [128, TB * 128], f32,
                           kind="ExternalInput").ap()
    sb_d = nc.dram_tensor("sb", [128, N], f16, kind="ExternalInput").ap()
    scols_d = nc.dram_tensor("scols", [128, TB], f16,
                             kind="ExternalInput").ap()
    tmask_d = nc.dram_tensor("tmask", [128, 2 * JW], f32,
                             kind="ExternalInput").ap()
    rowp_d = nc.dram_tensor("rowp", [TB * 128, 1], f32,
                            kind="ExternalOutput").ap()
    colp_d = nc.dram_tensor("colp", [1, N], f32, kind="ExternalOutput").ap()

    with tile.TileContext(nc) as tc:
        with ExitStack() as ctx:
            nc2 = tc.nc
            cst = ctx.enter_context(tc.tile_pool(name="cst", bufs=1))
            gp = ctx.enter_context(tc.tile_pool(name="gp", bufs=6,
                                                space="PSUM"))
            cpp = ctx.enter_context(tc.tile_pool(name="cpp", bufs=2,
                                                 space="PSUM"))
            sqp = ctx.enter_context(tc.tile_pool(name="sqp", bufs=4))
            mkp = ctx.enter_context(tc.tile_pool(name="mkp", bufs=17))
            s2p = ctx.enter_context(tc.tile_pool(name="s2p", bufs=3))
            fin = ctx.enter_context(tc.tile_pool(name="finp", bufs=2))

            xts_t = cst.tile([128, TB * 128], f32, tag="xts")
            nc2.sync.dma_start(out=xts_t[:], in_=xts_d[:])
            scols_t = cst.tile([128, TB], f16, tag="scols")
            nc2.sync.dma_start(out=scols_t[:], in_=scols_d[:])
            tmask_t = cst.tile([128, 2 * JW], f32, tag="tmask")
            nc2.sync.dma_start(out=tmask_t[:], in_=tmask_d[:])
            xt_tiles, sb_tiles = [], []
            for j in range(JT):
                t_ = cst.tile([128, JW], f32, tag=f"xt{j}")
                nc2.sync.dma_start(out=t_[:], in_=xt_d[:, j * JW:(j + 1) * JW])
                xt_tiles.append(t_)
                u = cst.tile([128, JW], f16, tag=f"sb{j}")
                nc2.sync.dma_start(out=u[:], in_=sb_d[:, j * JW:(j + 1) * JW])
                sb_tiles.append(u)

            # row-partial accumulators per owned block (slot per jt)
            aggRT = [cst.tile([128, JT], f32, tag=f"agr{t}",
                               name=f"agr{t}")
                     for t in range(TB)]

            th = float(np.float32(THRESH))

            def body(tag=""):
                for jt in range(JT):
                    t_end = jt // 2 + 1
                    ts = list(range(min(t_end, TB)))
                    colps = cpp.tile([1, JW], f32, tag="colps",
                                     name=f"colps_{tag}_{jt}")
                    pend = []

                    def emit_colp(item):
                        t_, mk_ = item
                        nc2.tensor.matmul(out=colps[:],
                                          lhsT=scols_t[:, t_:t_ + 1],
                                          rhs=mk_[:],
                                          start=(t_ == ts[0]),
                                          stop=(t_ == ts[-1]))

                    for t in ts:
                        lhsT = xts_t[:, t * 128:(t + 1) * 128]
                        g = gp.tile([128, JW], f32, tag="g")
                        nc2.tensor.matmul(out=g[:], lhsT=lhsT,
                                          rhs=xt_tiles[jt][:],
                                          start=True, stop=True)
                        sq = sqp.tile([128, JW], f32, tag="sq")
                        nc2.scalar.activation(sq[:], g[:], AFT.Square)
                        if jt == 2 * t or jt == 2 * t + 1:
                            slot = jt - 2 * t
                            sqm = sqp.tile([128, JW], f32, tag="sqm")
                            nc2.vector.tensor_tensor(
                                sqm[:], sq[:],
                                tmask_t[:, slot * JW:(slot + 1) * JW],
                                AOP.mult)
                            sq = sqm
                        mk = mkp.tile([128, JW], f16, tag="mk")
                        nc2.vector.tensor_scalar(mk[:], sq[:], th, None,
                                                 AOP.is_ge)
                        srt = s2p.tile([128, JW], f16, tag="srt")
                        nc2.vector.scalar_tensor_tensor(
                            out=srt[:], in0=mk[:], scalar=1.0,
                            in1=sb_tiles[jt][:],
                            op0=AOP.mult, op1=AOP.mult,
                            accum_out=aggRT[t][:, jt:jt + 1])
                        pend.append((t, mk))
                    for item in pend:
                        emit_colp(item)
                    cc = fin.tile([1, JW], f32, tag="cc",
                                  name=f"cc_{tag}_{jt}")
                    nc2.scalar.copy(cc[:], colps[:])
                    nc2.sync.dma_start(
                        out=colp_d[0:1, jt * JW:(jt + 1) * JW], in_=cc[:])

                for t in range(TB):
                    lo = 2 * t
                    agg = fin.tile([128, 1], f32, tag="agg",
                                   name=f"agg_{tag}_{t}")
                    nc2.vector.tensor_reduce(agg[:], aggRT[t][:, lo:JT],
                                             axis=AXL.X, op=AOP.add)
                    nc2.sync.dma_start(out=rowp_d[t * 128:(t + 1) * 128, :],
                                       in_=agg[:])

            if hwloop > 0:
                with tc.For_i(0, hwloop):
                    body("L")
            else:
                for rep in range(repeat):
                    body(str(rep))
    nc.compile()
    return nc


_CACHE = {}


def prep_in_maps(x, W, b):
    x = np.asarray(x, dtype=np.float32)
    xt = np.ascontiguousarray(x.T)                       # [128, N]
    s16 = (x.astype(np.float64).sum(axis=1) / 128.0).astype(np.float16)
    sb = np.ascontiguousarray(np.broadcast_to(s16[None, :], (128, N)))

    in_maps = []
    for c in range(NCORES):
        blocks = [8 * t + c for t in range(TB)]
        cols = np.concatenate([np.arange(k * 128, (k + 1) * 128)
                               for k in blocks])
        xts = np.ascontiguousarray(xt[:, cols])          # [128, 2048]
        scols = np.ascontiguousarray(
            s16[cols].reshape(TB, 128).T)                # [128, TB] f16
        # tmask slots: tile (t, jt=2t) keeps q >= 128c + p;
        #              tile (t, jt=2t+1) keeps q >= 128c - 512 + p
        p = np.arange(128)[:, None]
        q = np.arange(JW)[None, :]
        tm0 = (q >= 128 * c + p).astype(np.float32)
        tm1 = (q >= 128 * c - 512 + p).astype(np.float32)
        tmask = np.ascontiguousarray(
            np.concatenate([tm0, tm1], axis=1))          # [128, 1024]
        in_maps.append({
            "xt": xt, "xts": xts, "sb": sb,
            "scols": scols, "tmask": tmask,
        })
    return in_maps


def kernel(x: np.ndarray, W: np.ndarray, b: np.ndarray,
           trace: bool = False, tmpdir: str | None = None):
    x = np.asarray(x, dtype=np.float32)
    W = np.asarray(W, dtype=np.float32)
    b = np.asarray(b, dtype=np.float32)
    in_maps = prep_in_maps(x, W, b)

    if "nc" not in _CACHE:
        _CACHE["nc"] = _build_kernel()
    nc = _CACHE["nc"]
    res = run_bass_kernel_spmd(nc, in_maps, list(range(NCORES)),
                               trace=trace, tmpdir=tmpdir)

    s16 = (x.astype(np.float64).sum(axis=1) / 128.0).astype(np.float16)
    agg = np.zeros(N, dtype=np.float64)
    for c in range(NCORES):
        r = res.results[c]
        blocks = [8 * t + c for t in range(TB)]
        rows = np.concatenate([np.arange(k * 128, (k + 1) * 128)
                               for k in blocks])
        agg[rows] += r["rowp"][:, 0].astype(np.float64)
        agg += r["colp"][0].astype(np.float64)
    # diagonal was counted once in the row half and once in the col half
    agg -= 2.0 * s16.astype(np.float64)
    wsum1 = W.astype(np.float64).sum(axis=1)
    out = (agg[:, None] * wsum1[None, :] + b.astype(np.float64)[None, :])
    if trace:
        kernel.last_results = res
    return out.astype(np.float32)
